# revision 13
# baseline (speedup 1.0000x reference)
"""FSMN memory block (strided dilated depthwise conv over time) on 8 trn2 cores.

out[b,t,d] = sum_k filt[k,d] * x[b, t + off_k - 20, d] + x[b,t,d]
  off_k in {0,2,..,18} (left), {20} (center), {21,23,..,29} (right)

Architecture (v2):
- Data-parallel over batch: 16 items -> 2 per core, identical SPMD program.
- Host pre-transposes to channel-major [b, d, t] bf16 with zero time-padding,
  so every DMA row is contiguous (4KB-class descriptors) and the device does
  NO transposes at all. Host transposes the bf16 result back and casts fp32.
- The 16 taps are split across engines, all in channel-major [d, t] layout
  where a tap is just a column-offset window:
    * PE: 11 taps as diag-weight matmuls (bf16, 1 cycle/col) accumulating
      in fp32 PSUM, 4 chunks of 512 columns, tap-outer so LDWEIGHTS dedupes.
    * DVE: 3 taps as fused scalar_tensor_tensor MACs (per-partition scalar
      filter) into an fp32 SBUF accumulator.
    * GpSimd: 2 taps likewise, then folds its accumulator into DVE's.
  Residual is folded into the center tap (weight 1+f) on PE.
- DVE merges per chunk: out_sb = psum + acc (bf16 out), store DMAs on the
  ACT ring write contiguous [128, 512] bf16 blocks, one output tensor per
  (batch, group, chunk) so every store has a single writer.
"""

import sys

for p in ("/opt/trn_rl_repo", "/opt/trn_rl_repo/concourse"):
    if p not in sys.path:
        sys.path.insert(0, p)

import ml_dtypes
import numpy as np

import concourse.bass as bass
import concourse.mybir as mybir
import concourse.bass_utils as _bass_utils
from concourse.bass_utils import run_bass_kernel_spmd
from concourse.tile import TileContext

# Problem constants (hardcoded per contract).
B, T, D = 16, 2000, 512
NCORES = 8
B_LOC = B // NCORES          # 2 batch items per core
P = 128                      # partitions
NG = D // P                  # 4 channel groups
NROUNDS = B_LOC * NG         # 8 (b, g) rounds per core
NTAPS = 16
OFFS = [2 * k for k in range(10)] + [20] + [21 + 2 * k for k in range(5)]
PADL = 20                    # left zero pad inside the padded time axis
TOUT = 2048                  # output padded time
CH = 512                     # time chunk per psum bank
NCHK = TOUT // CH            # 4 chunks
TP = TOUT + 32               # input padded time (max window 29+2048)
F32 = mybir.dt.float32
BF16 = mybir.dt.bfloat16
NPBF16 = ml_dtypes.bfloat16

# Engine tap assignment (tap indices into OFFS). Center tap (10) carries the
# residual, keep it on PE where it accumulates in fp32 PSUM.
DVE_TAPS = [0, 1]
ACT_TAPS = [2, 3, 4]
PE_TAPS = [k for k in range(NTAPS) if k not in DVE_TAPS and k not in ACT_TAPS]
NV = len(DVE_TAPS) + len(ACT_TAPS)
NPE = len(PE_TAPS)

_CACHE = {}


def _build_bass(waitfix: bool = True):
    nc = bass.Bass()
    x = nc.declare_dram_parameter("x", [B_LOC, D, TP], BF16, isOutput=False)
    dw = nc.declare_dram_parameter("dw", [P, NPE, NG, P], BF16, isOutput=False)
    fv = nc.declare_dram_parameter("fv", [P, NV, NG], F32, isOutput=False)
    youts = {
        (b, g, c): nc.declare_dram_parameter(
            f"y_{b}_{g}_{c}", [P, CH], BF16, isOutput=True
        )
        for b in range(B_LOC)
        for g in range(NG)
        for c in range(NCHK)
    }

    with TileContext(nc) as tc:
        with (
            tc.tile_pool(name="wpool", bufs=1) as wpool,
            tc.tile_pool(name="xpool", bufs=NROUNDS) as xpool,
            tc.tile_pool(name="accp", bufs=2) as acc_pool,
            tc.tile_pool(name="outp", bufs=2) as out_pool,
            tc.tile_pool(name="psum", bufs=8, space="PSUM") as ps_pool,
        ):
            # All input tiles up front: 8 x [128, TP] bf16 loads, contiguous
            # rows, descriptors spread across the DMA queues. First two tiles
            # land before the weight DMA so round 0 starts ASAP.
            xts = {}
            for r in range(NROUNDS):
                xts[r] = xpool.tile([P, TP], BF16, name="xt")

            def load_xt(r):
                b, g = divmod(r, NG)
                nc.sync.dma_start(out=xts[r], in_=x[b, g * P : (g + 1) * P, :])

            load_xt(0)
            load_xt(1)
            fvt = wpool.tile([P, NV, NG], F32, name="fvt")
            nc.sync.dma_start(out=fvt, in_=fv[:, :, :])
            wt = wpool.tile([P, NPE, NG, P], BF16, name="wt")
            nc.sync.dma_start(out=wt, in_=dw[:, :, :, :])
            for r in range(2, NROUNDS):
                load_xt(r)

            for r in range(NROUNDS):
                b, g = divmod(r, NG)
                xt = xts[r]

                # ---- Act taps: per-partition-scaled copies (bf16 partials) ----
                pacts = []
                for ai, k in enumerate(ACT_TAPS):
                    vi = len(DVE_TAPS) + ai
                    pa = acc_pool.tile([P, TOUT], BF16, name=f"pact{ai}")
                    nc.scalar.mul(
                        pa, xt[:, OFFS[k] : OFFS[k] + TOUT], fvt[:, vi, g : g + 1]
                    )
                    pacts.append(pa)

                # ---- DVE taps: 4x-mode mult + 2x-mode add, bf16 ----
                acc = acc_pool.tile([P, TOUT], BF16, name="acc")
                tmp = acc_pool.tile([P, TOUT], BF16, name="tmp")
                for vi, k in enumerate(DVE_TAPS):
                    w = xt[:, OFFS[k] : OFFS[k] + TOUT]
                    if vi == 0:
                        nc.vector.tensor_scalar(
                            acc, w, fvt[:, vi, g : g + 1], None, mybir.AluOpType.mult
                        )
                    else:
                        nc.vector.tensor_scalar(
                            tmp, w, fvt[:, vi, g : g + 1], None, mybir.AluOpType.mult
                        )
                        nc.vector.tensor_tensor(acc, acc, tmp, mybir.AluOpType.add)
                # Fold the Act partials.
                for pa in pacts:
                    nc.vector.tensor_tensor(acc, acc, pa, mybir.AluOpType.add)

                # ---- PE taps: tap-outer over 4 psum chunks ----
                pss = [
                    ps_pool.tile([P, CH], F32, name="ps") for _ in range(NCHK)
                ]
                for ki, k in enumerate(PE_TAPS):
                    for c in range(NCHK):
                        nc.tensor.matmul(
                            pss[c],
                            wt[:, ki, g, :],
                            xt[:, c * CH + OFFS[k] : c * CH + OFFS[k] + CH],
                            start=(ki == 0),
                            stop=(ki == NPE - 1),
                            skip_group_check=True,
                        )

                # ---- merge + store per chunk ----
                out_sb = out_pool.tile([P, TOUT], BF16, name="out_sb")
                for c in range(NCHK):
                    nc.vector.scalar_tensor_tensor(
                        out_sb[:, c * CH : (c + 1) * CH],
                        pss[c], 1.0, acc[:, c * CH : (c + 1) * CH],
                        mybir.AluOpType.mult, mybir.AluOpType.add,
                    )
                    nc.sync.dma_start(
                        out=youts[(b, g, c)][:, :],
                        in_=out_sb[:, c * CH : (c + 1) * CH],
                    )

    # The tile legalizer emits one LDWEIGHTS per bf16 matmul; with tap-outer
    # ordering the 4 chunk matmuls of one tap reload identical weights
    # (~98ns each on the PE pipe). Drop the duplicates, migrating their waits
    # to the next PE-queue instruction.
    PE_ENG = mybir.EngineType.PE
    for fn in nc.m.functions:
        for blk in fn.blocks:
            out_insts = []
            last_key = None
            pending = []
            for inst in blk.instructions:
                tn = type(inst).__name__
                if getattr(inst, "engine", None) == PE_ENG or tn in (
                    "InstLdweights",
                    "InstMatmult",
                ):
                    if tn == "InstLdweights":
                        w = inst.ins[0]
                        key = (
                            w.memref,
                            w.offset,
                            str(w.ap),
                            str(w.dtype),
                            str(inst.perf_mode),
                            str(inst.is_transpose),
                        )
                        if key == last_key:
                            si = inst.sync_info
                            if si is not None:
                                assert not si.on_update, inst.name
                                pending.extend(si.on_wait)
                            continue  # drop duplicate
                        last_key = key
                    elif tn == "InstMatmult":
                        pass  # legalized matmuls don't clobber the array
                    else:
                        last_key = None  # unknown PE op: be conservative
                    if pending:
                        si = inst.sync_info
                        ow = list(si.on_wait) if si else []
                        ou = list(si.on_update) if si else []
                        inst.sync_info = mybir.SyncInfo(
                            on_wait=pending + ow, on_update=ou
                        )
                        pending = []
                out_insts.append(inst)
            assert not pending
            blk.instructions = out_insts

    # TRN2 ISA structs encode a single sync-wait. Split every multi-wait
    # instruction: single-wait NoOps on the same queue immediately before it
    # carry the extra waits (the sequencer blocks on each in order).
    if not waitfix:
        return nc
    nfix = [0]
    for fn in nc.m.functions:
        for blk in fn.blocks:
            out_insts = []
            for inst in blk.instructions:
                si = inst.sync_info
                if si is not None and len(si.on_wait) > 1:
                    w = list(si.on_wait)
                    for wt_ in w[:-1]:
                        nop = mybir.InstNoOp(name=f"waitfix_{nfix[0]}")
                        nfix[0] += 1
                        nop.engine = inst.engine
                        nop.sync_info = mybir.SyncInfo(
                            on_wait=[wt_], on_update=[]
                        )
                        out_insts.append(nop)
                    inst.sync_info = mybir.SyncInfo(
                        on_wait=[w[-1]], on_update=list(si.on_update)
                    )
                out_insts.append(inst)
            blk.instructions = out_insts
    return nc


def _pack_weights(filt: np.ndarray):
    fw = filt.astype(np.float32).copy()
    fw[10] += 1.0  # fold the residual into the center tap
    dwm = np.zeros((P, NPE, NG, P), np.float32)
    for ki, k in enumerate(PE_TAPS):
        for g in range(NG):
            dwm[np.arange(P), ki, g, np.arange(P)] = fw[k, g * P : (g + 1) * P]
    fvm = np.zeros((P, NV, NG), np.float32)
    for vi, k in enumerate(DVE_TAPS + ACT_TAPS):
        for g in range(NG):
            fvm[:, vi, g] = fw[k, g * P : (g + 1) * P]
    return dwm.astype(NPBF16), fvm


def kernel(inputs: np.ndarray, filt: np.ndarray, _trace: bool = False):
    inputs = np.asarray(inputs, dtype=np.float32)
    filt = np.asarray(filt, dtype=np.float32)

    # Channel-major, zero-padded, bf16.
    xp = np.zeros((B, D, TP), NPBF16)
    xp[:, :, PADL : PADL + T] = inputs.transpose(0, 2, 1).astype(NPBF16)
    dwm, fvm = _pack_weights(filt)
    in_maps = [
        {"x": xp[c * B_LOC : (c + 1) * B_LOC], "dw": dwm, "fv": fvm}
        for c in range(NCORES)
    ]

    if "nc" not in _CACHE:
        _CACHE["nc"] = _build_bass()
    nc = _CACHE["nc"]
    res = run_bass_kernel_spmd(nc, in_maps, list(range(NCORES)), trace=_trace)
    ycm = np.empty((B, D, TOUT), NPBF16)
    for core in range(NCORES):
        r = res.results[core]
        for b in range(B_LOC):
            for g in range(NG):
                for c in range(NCHK):
                    ycm[core * B_LOC + b, g * P : (g + 1) * P,
                        c * CH : (c + 1) * CH] = np.asarray(r[f"y_{b}_{g}_{c}"])
    out = np.ascontiguousarray(
        ycm[:, :, :T].transpose(0, 2, 1)
    ).astype(np.float32)
    if _trace:
        return out, res
    return out


if __name__ == "__main__":
    rng = np.random.default_rng(0)
    xs = rng.standard_normal((B, T, D), dtype=np.float32)
    ft = rng.standard_normal((NTAPS, D), dtype=np.float32)
    out = kernel(xs, ft)
    print("ran ok", out.shape, out.dtype)


# revision 15
# speedup vs baseline: 1.0810x; 1.0810x over previous
"""FSMN memory block (strided dilated depthwise conv over time) on 8 trn2 cores.

out[b,t,d] = sum_k filt[k,d] * x[b, t + off_k - 20, d] + x[b,t,d]
  off_k in {0,2,..,18} (left), {20} (center), {21,23,..,29} (right)

Architecture (v2):
- Data-parallel over batch: 16 items -> 2 per core, identical SPMD program.
- Host pre-transposes to channel-major [b, d, t] bf16 with zero time-padding,
  so every DMA row is contiguous (4KB-class descriptors) and the device does
  NO transposes at all. Host transposes the bf16 result back and casts fp32.
- The 16 taps are split across engines, all in channel-major [d, t] layout
  where a tap is just a column-offset window:
    * PE: 11 taps as diag-weight matmuls (bf16, 1 cycle/col) accumulating
      in fp32 PSUM, 4 chunks of 512 columns, tap-outer so LDWEIGHTS dedupes.
    * DVE: 3 taps as fused scalar_tensor_tensor MACs (per-partition scalar
      filter) into an fp32 SBUF accumulator.
    * GpSimd: 2 taps likewise, then folds its accumulator into DVE's.
  Residual is folded into the center tap (weight 1+f) on PE.
- DVE merges per chunk: out_sb = psum + acc (bf16 out), store DMAs on the
  ACT ring write contiguous [128, 512] bf16 blocks, one output tensor per
  (batch, group, chunk) so every store has a single writer.
"""

import sys

for p in ("/opt/trn_rl_repo", "/opt/trn_rl_repo/concourse"):
    if p not in sys.path:
        sys.path.insert(0, p)

import ml_dtypes
import numpy as np

import concourse.bass as bass
import concourse.mybir as mybir
import concourse.bass_utils as _bass_utils
from concourse.bass_utils import run_bass_kernel_spmd
from concourse.tile import TileContext

# Problem constants (hardcoded per contract).
B, T, D = 16, 2000, 512
NCORES = 8
B_LOC = B // NCORES          # 2 batch items per core
P = 128                      # partitions
NG = D // P                  # 4 channel groups
NROUNDS = B_LOC * NG         # 8 (b, g) rounds per core
NTAPS = 16
OFFS = [2 * k for k in range(10)] + [20] + [21 + 2 * k for k in range(5)]
PADL = 20                    # left zero pad inside the padded time axis
TOUT = 2000                  # output time (exactly T; host drops nothing)
CH = 500                     # time chunk per psum bank
NCHK = TOUT // CH            # 4 chunks
TP = TOUT + 32               # input padded time (max window 29+2048)
F32 = mybir.dt.float32
BF16 = mybir.dt.bfloat16
NPBF16 = ml_dtypes.bfloat16

# Engine tap assignment (tap indices into OFFS). Center tap (10) carries the
# residual, keep it on PE where it accumulates in fp32 PSUM.
DVE_TAPS = [0, 1, 2]
ACT_TAPS = [3, 4, 5]
PE_TAPS = [k for k in range(NTAPS) if k not in DVE_TAPS and k not in ACT_TAPS]
NV = len(DVE_TAPS) + len(ACT_TAPS)
NPE = len(PE_TAPS)

_CACHE = {}


def _build_bass(waitfix: bool = True):
    nc = bass.Bass()
    x = nc.declare_dram_parameter("x", [B_LOC, D, TP], BF16, isOutput=False)
    dw = nc.declare_dram_parameter("dw", [P, NPE, NG, P], BF16, isOutput=False)
    fv = nc.declare_dram_parameter("fv", [P, NV, NG], F32, isOutput=False)
    youts = {
        (b, g, c): nc.declare_dram_parameter(
            f"y_{b}_{g}_{c}", [P, CH], BF16, isOutput=True
        )
        for b in range(B_LOC)
        for g in range(NG)
        for c in range(NCHK)
    }

    with TileContext(nc) as tc:
        with (
            tc.tile_pool(name="wpool", bufs=1) as wpool,
            tc.tile_pool(name="xpool", bufs=NROUNDS) as xpool,
            tc.tile_pool(name="accp", bufs=2) as acc_pool,
            tc.tile_pool(name="outp", bufs=2) as out_pool,
            tc.tile_pool(name="psum", bufs=7, space="PSUM") as ps_pool,
        ):
            # All input tiles up front: 8 x [128, TP] bf16 loads, contiguous
            # rows, descriptors spread across the DMA queues. First two tiles
            # land before the weight DMA so round 0 starts ASAP.
            xts = {}
            for r in range(NROUNDS):
                xts[r] = xpool.tile([P, TP], BF16, name="xt")

            def load_xt(r):
                b, g = divmod(r, NG)
                nc.sync.dma_start(out=xts[r], in_=x[b, g * P : (g + 1) * P, :])

            fvt = wpool.tile([P, NV, NG], F32, name="fvt")
            nc.sync.dma_start(out=fvt, in_=fv[:, :, :])
            load_xt(0)
            wt = wpool.tile([P, NPE, NG, P], BF16, name="wt")
            nc.sync.dma_start(out=wt[:, 0:1], in_=dw[:, 0:1])
            load_xt(1)
            nc.sync.dma_start(out=wt[:, 1:4], in_=dw[:, 1:4])
            nc.sync.dma_start(out=wt[:, 4:NPE], in_=dw[:, 4:NPE])
            for r in range(2, NROUNDS):
                load_xt(r)

            # PE p-state warmup: a dependency-free junk stream keeps the PE
            # clock ramping while the first loads land. Reads whatever is in
            # the xt tiles (values never consumed), accumulates in one junk
            # psum cell.
            junk = ps_pool.tile([P, CH], F32, name="junkps", bufs=1)
            for _ in range(40):
                nc.tensor.matmul(
                    junk[0:1, 0:64],
                    xts[0][0:1, 0:1],
                    xts[0][0:1, 0:64],
                    start=True, stop=True, skip_group_check=True,
                )

            for r in range(NROUNDS):
                b, g = divmod(r, NG)
                xt = xts[r]

                # ---- Act taps: per-partition-scaled copies (bf16 partials) ----
                pacts = []
                for ai, k in enumerate(ACT_TAPS):
                    vi = len(DVE_TAPS) + ai
                    pa = acc_pool.tile([P, TOUT], BF16, name=f"pact{ai}")
                    nc.scalar.mul(
                        pa, xt[:, OFFS[k] : OFFS[k] + TOUT], fvt[:, vi, g : g + 1]
                    )
                    pacts.append(pa)

                # ---- DVE taps: 4x-mode mult + 2x-mode add, bf16 ----
                acc = acc_pool.tile([P, TOUT], BF16, name="acc")
                tmp = acc_pool.tile([P, TOUT], BF16, name="tmp")
                for vi, k in enumerate(DVE_TAPS):
                    w = xt[:, OFFS[k] : OFFS[k] + TOUT]
                    if vi == 0:
                        nc.vector.tensor_scalar(
                            acc, w, fvt[:, vi, g : g + 1], None, mybir.AluOpType.mult
                        )
                    else:
                        nc.vector.tensor_scalar(
                            tmp, w, fvt[:, vi, g : g + 1], None, mybir.AluOpType.mult
                        )
                        nc.vector.tensor_tensor(acc, acc, tmp, mybir.AluOpType.add)
                # Fold the Act partials.
                for pa in pacts:
                    nc.vector.tensor_tensor(acc, acc, pa, mybir.AluOpType.add)

                # ---- PE taps: tap-outer over 4 psum chunks ----
                pss = [
                    ps_pool.tile([P, CH], F32, name="ps") for _ in range(NCHK)
                ]
                for ki, k in enumerate(PE_TAPS):
                    for c in range(NCHK):
                        nc.tensor.matmul(
                            pss[c],
                            wt[:, ki, g, :],
                            xt[:, c * CH + OFFS[k] : c * CH + OFFS[k] + CH],
                            start=(ki == 0),
                            stop=(ki == NPE - 1),
                            skip_group_check=True,
                        )

                # ---- evacuate + merge + store ----
                # Act copies each psum chunk to bf16 (fast-path frees the
                # psum bank); DVE adds the elementwise accumulator in one
                # 2x-mode pass; stores go per chunk on the sync ring.
                psb = out_pool.tile([P, TOUT], BF16, name="psb")
                for c in range(NCHK):
                    nc.scalar.copy(psb[:, c * CH : (c + 1) * CH], pss[c])
                out_sb = out_pool.tile([P, TOUT], BF16, name="out_sb")
                nc.vector.tensor_tensor(out_sb, psb, acc, mybir.AluOpType.add)
                for c in range(NCHK):
                    nc.sync.dma_start(
                        out=youts[(b, g, c)][:, :],
                        in_=out_sb[:, c * CH : (c + 1) * CH],
                    )

    # The tile legalizer emits one LDWEIGHTS per bf16 matmul; with tap-outer
    # ordering the 4 chunk matmuls of one tap reload identical weights
    # (~98ns each on the PE pipe). Drop the duplicates, migrating their waits
    # to the next PE-queue instruction.
    PE_ENG = mybir.EngineType.PE
    for fn in nc.m.functions:
        for blk in fn.blocks:
            out_insts = []
            last_key = None
            pending = []
            for inst in blk.instructions:
                tn = type(inst).__name__
                if getattr(inst, "engine", None) == PE_ENG or tn in (
                    "InstLdweights",
                    "InstMatmult",
                ):
                    if tn == "InstLdweights":
                        w = inst.ins[0]
                        key = (
                            w.memref,
                            w.offset,
                            str(w.ap),
                            str(w.dtype),
                            str(inst.perf_mode),
                            str(inst.is_transpose),
                        )
                        if key == last_key:
                            si = inst.sync_info
                            if si is not None:
                                assert not si.on_update, inst.name
                                pending.extend(si.on_wait)
                            continue  # drop duplicate
                        last_key = key
                    elif tn == "InstMatmult":
                        pass  # legalized matmuls don't clobber the array
                    else:
                        last_key = None  # unknown PE op: be conservative
                    if pending:
                        si = inst.sync_info
                        ow = list(si.on_wait) if si else []
                        ou = list(si.on_update) if si else []
                        inst.sync_info = mybir.SyncInfo(
                            on_wait=pending + ow, on_update=ou
                        )
                        pending = []
                out_insts.append(inst)
            assert not pending
            blk.instructions = out_insts

    # TRN2 ISA structs encode a single sync-wait. Split every multi-wait
    # instruction: single-wait NoOps on the same queue immediately before it
    # carry the extra waits (the sequencer blocks on each in order).
    if not waitfix:
        return nc
    nfix = [0]
    for fn in nc.m.functions:
        for blk in fn.blocks:
            out_insts = []
            for inst in blk.instructions:
                si = inst.sync_info
                if si is not None and len(si.on_wait) > 1:
                    w = list(si.on_wait)
                    for wt_ in w[:-1]:
                        nop = mybir.InstNoOp(name=f"waitfix_{nfix[0]}")
                        nfix[0] += 1
                        nop.engine = inst.engine
                        nop.sync_info = mybir.SyncInfo(
                            on_wait=[wt_], on_update=[]
                        )
                        out_insts.append(nop)
                    inst.sync_info = mybir.SyncInfo(
                        on_wait=[w[-1]], on_update=list(si.on_update)
                    )
                out_insts.append(inst)
            blk.instructions = out_insts
    return nc


def _pack_weights(filt: np.ndarray):
    fw = filt.astype(np.float32).copy()
    fw[10] += 1.0  # fold the residual into the center tap
    dwm = np.zeros((P, NPE, NG, P), np.float32)
    for ki, k in enumerate(PE_TAPS):
        for g in range(NG):
            dwm[np.arange(P), ki, g, np.arange(P)] = fw[k, g * P : (g + 1) * P]
    fvm = np.zeros((P, NV, NG), np.float32)
    for vi, k in enumerate(DVE_TAPS + ACT_TAPS):
        for g in range(NG):
            fvm[:, vi, g] = fw[k, g * P : (g + 1) * P]
    return dwm.astype(NPBF16), fvm


def kernel(inputs: np.ndarray, filt: np.ndarray, _trace: bool = False):
    inputs = np.asarray(inputs, dtype=np.float32)
    filt = np.asarray(filt, dtype=np.float32)

    # Channel-major, zero-padded, bf16.
    xp = np.zeros((B, D, TP), NPBF16)
    xp[:, :, PADL : PADL + T] = inputs.transpose(0, 2, 1).astype(NPBF16)
    dwm, fvm = _pack_weights(filt)
    in_maps = [
        {"x": xp[c * B_LOC : (c + 1) * B_LOC], "dw": dwm, "fv": fvm}
        for c in range(NCORES)
    ]

    if "nc" not in _CACHE:
        _CACHE["nc"] = _build_bass()
    nc = _CACHE["nc"]
    res = run_bass_kernel_spmd(nc, in_maps, list(range(NCORES)), trace=_trace)
    ycm = np.empty((B, D, TOUT), NPBF16)
    for core in range(NCORES):
        r = res.results[core]
        for b in range(B_LOC):
            for g in range(NG):
                for c in range(NCHK):
                    ycm[core * B_LOC + b, g * P : (g + 1) * P,
                        c * CH : (c + 1) * CH] = np.asarray(r[f"y_{b}_{g}_{c}"])
    out = np.ascontiguousarray(
        ycm[:, :, :T].transpose(0, 2, 1)
    ).astype(np.float32)
    if _trace:
        return out, res
    return out


if __name__ == "__main__":
    rng = np.random.default_rng(0)
    xs = rng.standard_normal((B, T, D), dtype=np.float32)
    ft = rng.standard_normal((NTAPS, D), dtype=np.float32)
    out = kernel(xs, ft)
    print("ran ok", out.shape, out.dtype)


# revision 16
# speedup vs baseline: 1.1441x; 1.0583x over previous
"""FSMN memory block (strided dilated depthwise conv over time) on 8 trn2 cores.

out[b,t,d] = sum_k filt[k,d] * x[b, t + off_k - 20, d] + x[b,t,d]
  off_k in {0,2,..,18} (left), {20} (center), {21,23,..,29} (right)

Architecture (v2):
- Data-parallel over batch: 16 items -> 2 per core, identical SPMD program.
- Host pre-transposes to channel-major [b, d, t] bf16 with zero time-padding,
  so every DMA row is contiguous (4KB-class descriptors) and the device does
  NO transposes at all. Host transposes the bf16 result back and casts fp32.
- The 16 taps are split across engines, all in channel-major [d, t] layout
  where a tap is just a column-offset window:
    * PE: 11 taps as diag-weight matmuls (bf16, 1 cycle/col) accumulating
      in fp32 PSUM, 4 chunks of 512 columns, tap-outer so LDWEIGHTS dedupes.
    * DVE: 3 taps as fused scalar_tensor_tensor MACs (per-partition scalar
      filter) into an fp32 SBUF accumulator.
    * GpSimd: 2 taps likewise, then folds its accumulator into DVE's.
  Residual is folded into the center tap (weight 1+f) on PE.
- DVE merges per chunk: out_sb = psum + acc (bf16 out), store DMAs on the
  ACT ring write contiguous [128, 512] bf16 blocks, one output tensor per
  (batch, group, chunk) so every store has a single writer.
"""

import sys

for p in ("/opt/trn_rl_repo", "/opt/trn_rl_repo/concourse"):
    if p not in sys.path:
        sys.path.insert(0, p)

import ml_dtypes
import numpy as np

import concourse.bass as bass
import concourse.mybir as mybir
import concourse.bass_utils as _bass_utils
from concourse.bass_utils import run_bass_kernel_spmd
from concourse.tile import TileContext

# Problem constants (hardcoded per contract).
B, T, D = 16, 2000, 512
NCORES = 8
B_LOC = B // NCORES          # 2 batch items per core
P = 128                      # partitions
NG = D // P                  # 4 channel groups
NROUNDS = B_LOC * NG         # 8 (b, g) rounds per core
NTAPS = 16
OFFS = [2 * k for k in range(10)] + [20] + [21 + 2 * k for k in range(5)]
PADL = 20                    # left zero pad inside the padded time axis
TOUT = 2000                  # output time (exactly T; host drops nothing)
CH = 500                     # time chunk per psum bank
NCHK = TOUT // CH            # 4 chunks
TP = TOUT + 32               # input padded time (max window 29+2048)
F32 = mybir.dt.float32
BF16 = mybir.dt.bfloat16
NPBF16 = ml_dtypes.bfloat16

# Engine tap assignment (tap indices into OFFS). Center tap (10) carries the
# residual, keep it on PE where it accumulates in fp32 PSUM.
DVE_TAPS = [0, 1, 2]
ACT_TAPS = [3, 4, 5]
PE_TAPS = [k for k in range(NTAPS) if k not in DVE_TAPS and k not in ACT_TAPS]
NV = len(DVE_TAPS) + len(ACT_TAPS)
NPE = len(PE_TAPS)

_CACHE = {}


def _build_bass(waitfix: bool = True):
    nc = bass.Bass()
    x = nc.declare_dram_parameter("x", [B_LOC, D, TP], BF16, isOutput=False)
    dw = nc.declare_dram_parameter("dw", [P, NPE, NG, P], BF16, isOutput=False)
    fv = nc.declare_dram_parameter("fv", [P, NV, NG], F32, isOutput=False)
    youts = {
        (b, g, c): nc.declare_dram_parameter(
            f"y_{b}_{g}_{c}", [P, CH], BF16, isOutput=True
        )
        for b in range(B_LOC)
        for g in range(NG)
        for c in range(NCHK)
    }

    with TileContext(nc) as tc:
        with (
            tc.tile_pool(name="wpool", bufs=1) as wpool,
            tc.tile_pool(name="xpool", bufs=NROUNDS) as xpool,
            tc.tile_pool(name="accp", bufs=2) as acc_pool,
            tc.tile_pool(name="outp", bufs=2) as out_pool,
            tc.tile_pool(name="psum", bufs=7, space="PSUM") as ps_pool,
        ):
            # All input tiles up front: 8 x [128, TP] bf16 loads, contiguous
            # rows, descriptors spread across the DMA queues. First two tiles
            # land before the weight DMA so round 0 starts ASAP.
            xts = {}
            for r in range(NROUNDS):
                xts[r] = xpool.tile([P, TP], BF16, name="xt")

            XSPLIT = 2 * CH + 32
            def load_xt(r):
                b, g = divmod(r, NG)
                nc.sync.dma_start(
                    out=xts[r][:, :XSPLIT],
                    in_=x[b, g * P : (g + 1) * P, :XSPLIT],
                )
                nc.sync.dma_start(
                    out=xts[r][:, XSPLIT:],
                    in_=x[b, g * P : (g + 1) * P, XSPLIT:],
                )

            fvt = wpool.tile([P, NV, NG], F32, name="fvt")
            nc.sync.dma_start(out=fvt, in_=fv[:, :, :])
            load_xt(0)
            wt = wpool.tile([P, NPE, NG, P], BF16, name="wt")
            nc.sync.dma_start(out=wt[:, 0:1], in_=dw[:, 0:1])
            load_xt(1)
            nc.sync.dma_start(out=wt[:, 1:4], in_=dw[:, 1:4])
            nc.sync.dma_start(out=wt[:, 4:NPE], in_=dw[:, 4:NPE])
            for r in range(2, NROUNDS):
                load_xt(r)

            # PE p-state warmup: a dependency-free junk stream keeps the PE
            # clock ramping while the first loads land. Reads whatever is in
            # the xt tiles (values never consumed), accumulates in one junk
            # psum cell.
            junk = ps_pool.tile([P, CH], F32, name="junkps", bufs=1)
            for _ in range(40):
                nc.tensor.matmul(
                    junk[0:1, 0:64],
                    xts[0][0:1, 0:1],
                    xts[0][0:1, 0:64],
                    start=True, stop=True, skip_group_check=True,
                )

            for r in range(NROUNDS):
                b, g = divmod(r, NG)
                xt = xts[r]

                # ---- Act taps: per-partition-scaled copies (bf16 partials) ----
                pacts = []
                for ai, k in enumerate(ACT_TAPS):
                    vi = len(DVE_TAPS) + ai
                    pa = acc_pool.tile([P, TOUT], BF16, name=f"pact{ai}")
                    nc.scalar.mul(
                        pa, xt[:, OFFS[k] : OFFS[k] + TOUT], fvt[:, vi, g : g + 1]
                    )
                    pacts.append(pa)

                # ---- DVE taps: 4x-mode mult + 2x-mode add, bf16 ----
                acc = acc_pool.tile([P, TOUT], BF16, name="acc")
                tmp = acc_pool.tile([P, TOUT], BF16, name="tmp")
                for vi, k in enumerate(DVE_TAPS):
                    w = xt[:, OFFS[k] : OFFS[k] + TOUT]
                    if vi == 0:
                        nc.vector.tensor_scalar(
                            acc, w, fvt[:, vi, g : g + 1], None, mybir.AluOpType.mult
                        )
                    else:
                        nc.vector.tensor_scalar(
                            tmp, w, fvt[:, vi, g : g + 1], None, mybir.AluOpType.mult
                        )
                        nc.vector.tensor_tensor(acc, acc, tmp, mybir.AluOpType.add)
                # Fold the Act partials.
                for pa in pacts:
                    nc.vector.tensor_tensor(acc, acc, pa, mybir.AluOpType.add)

                # ---- PE taps: chunk-major so chunk c is evacuable after
                # NPE matmuls (LDWEIGHTS overlaps the array stream, so the
                # extra per-chunk weight reloads are free) ----
                pss = [
                    ps_pool.tile([P, CH], F32, name="ps") for _ in range(NCHK)
                ]
                psb = out_pool.tile([P, TOUT], BF16, name="psb")
                for c in range(NCHK):
                    for ki, k in enumerate(PE_TAPS):
                        nc.tensor.matmul(
                            pss[c],
                            wt[:, ki, g, :],
                            xt[:, c * CH + OFFS[k] : c * CH + OFFS[k] + CH],
                            start=(ki == 0),
                            stop=(ki == NPE - 1),
                            skip_group_check=True,
                        )
                    # Act evacuates the finished chunk to bf16 (frees bank).
                    nc.scalar.copy(psb[:, c * CH : (c + 1) * CH], pss[c])

                # ---- merge + store ----
                out_sb = out_pool.tile([P, TOUT], BF16, name="out_sb")
                if r < NROUNDS - 1:
                    nc.vector.tensor_tensor(
                        out_sb, psb, acc, mybir.AluOpType.add
                    )
                    for c in range(NCHK):
                        nc.sync.dma_start(
                            out=youts[(b, g, c)][:, :],
                            in_=out_sb[:, c * CH : (c + 1) * CH],
                        )
                else:
                    # Last round: per-chunk merge + store to shorten the tail.
                    for c in range(NCHK):
                        sl = slice(c * CH, (c + 1) * CH)
                        nc.vector.tensor_tensor(
                            out_sb[:, sl], psb[:, sl], acc[:, sl],
                            mybir.AluOpType.add,
                        )
                        nc.sync.dma_start(
                            out=youts[(b, g, c)][:, :], in_=out_sb[:, sl]
                        )

    # The tile legalizer emits one LDWEIGHTS per bf16 matmul; with tap-outer
    # ordering the 4 chunk matmuls of one tap reload identical weights
    # (~98ns each on the PE pipe). Drop the duplicates, migrating their waits
    # to the next PE-queue instruction.
    PE_ENG = mybir.EngineType.PE
    for fn in nc.m.functions:
        for blk in fn.blocks:
            out_insts = []
            last_key = None
            pending = []
            for inst in blk.instructions:
                tn = type(inst).__name__
                if getattr(inst, "engine", None) == PE_ENG or tn in (
                    "InstLdweights",
                    "InstMatmult",
                ):
                    if tn == "InstLdweights":
                        w = inst.ins[0]
                        key = (
                            w.memref,
                            w.offset,
                            str(w.ap),
                            str(w.dtype),
                            str(inst.perf_mode),
                            str(inst.is_transpose),
                        )
                        if key == last_key:
                            si = inst.sync_info
                            if si is not None:
                                assert not si.on_update, inst.name
                                pending.extend(si.on_wait)
                            continue  # drop duplicate
                        last_key = key
                    elif tn == "InstMatmult":
                        pass  # legalized matmuls don't clobber the array
                    else:
                        last_key = None  # unknown PE op: be conservative
                    if pending:
                        si = inst.sync_info
                        ow = list(si.on_wait) if si else []
                        ou = list(si.on_update) if si else []
                        inst.sync_info = mybir.SyncInfo(
                            on_wait=pending + ow, on_update=ou
                        )
                        pending = []
                out_insts.append(inst)
            assert not pending
            blk.instructions = out_insts

    # TRN2 ISA structs encode a single sync-wait. Split every multi-wait
    # instruction: single-wait NoOps on the same queue immediately before it
    # carry the extra waits (the sequencer blocks on each in order).
    if not waitfix:
        return nc
    nfix = [0]
    for fn in nc.m.functions:
        for blk in fn.blocks:
            out_insts = []
            for inst in blk.instructions:
                si = inst.sync_info
                if si is not None and len(si.on_wait) > 1:
                    w = list(si.on_wait)
                    for wt_ in w[:-1]:
                        nop = mybir.InstNoOp(name=f"waitfix_{nfix[0]}")
                        nfix[0] += 1
                        nop.engine = inst.engine
                        nop.sync_info = mybir.SyncInfo(
                            on_wait=[wt_], on_update=[]
                        )
                        out_insts.append(nop)
                    inst.sync_info = mybir.SyncInfo(
                        on_wait=[w[-1]], on_update=list(si.on_update)
                    )
                out_insts.append(inst)
            blk.instructions = out_insts
    return nc


def _pack_weights(filt: np.ndarray):
    fw = filt.astype(np.float32).copy()
    fw[10] += 1.0  # fold the residual into the center tap
    dwm = np.zeros((P, NPE, NG, P), np.float32)
    for ki, k in enumerate(PE_TAPS):
        for g in range(NG):
            dwm[np.arange(P), ki, g, np.arange(P)] = fw[k, g * P : (g + 1) * P]
    fvm = np.zeros((P, NV, NG), np.float32)
    for vi, k in enumerate(DVE_TAPS + ACT_TAPS):
        for g in range(NG):
            fvm[:, vi, g] = fw[k, g * P : (g + 1) * P]
    return dwm.astype(NPBF16), fvm


def kernel(inputs: np.ndarray, filt: np.ndarray, _trace: bool = False):
    inputs = np.asarray(inputs, dtype=np.float32)
    filt = np.asarray(filt, dtype=np.float32)

    # Channel-major, zero-padded, bf16.
    xp = np.zeros((B, D, TP), NPBF16)
    xp[:, :, PADL : PADL + T] = inputs.transpose(0, 2, 1).astype(NPBF16)
    dwm, fvm = _pack_weights(filt)
    in_maps = [
        {"x": xp[c * B_LOC : (c + 1) * B_LOC], "dw": dwm, "fv": fvm}
        for c in range(NCORES)
    ]

    if "nc" not in _CACHE:
        _CACHE["nc"] = _build_bass()
    nc = _CACHE["nc"]
    res = run_bass_kernel_spmd(nc, in_maps, list(range(NCORES)), trace=_trace)
    ycm = np.empty((B, D, TOUT), NPBF16)
    for core in range(NCORES):
        r = res.results[core]
        for b in range(B_LOC):
            for g in range(NG):
                for c in range(NCHK):
                    ycm[core * B_LOC + b, g * P : (g + 1) * P,
                        c * CH : (c + 1) * CH] = np.asarray(r[f"y_{b}_{g}_{c}"])
    out = np.ascontiguousarray(
        ycm[:, :, :T].transpose(0, 2, 1)
    ).astype(np.float32)
    if _trace:
        return out, res
    return out


if __name__ == "__main__":
    rng = np.random.default_rng(0)
    xs = rng.standard_normal((B, T, D), dtype=np.float32)
    ft = rng.standard_normal((NTAPS, D), dtype=np.float32)
    out = kernel(xs, ft)
    print("ran ok", out.shape, out.dtype)
